# revision 10
# baseline (speedup 1.0000x reference)
import sys
sys.path.insert(0, '/opt/trn_rl_repo')
import numpy as np

P = 128
NCORES = 8
F = 128
SLICE = 12544          # rows per core (98 blocks)
NBLK = 98
QBLK = [25, 25, 24, 24]            # blocks per quarter
QB0 = [0, 25, 50, 74, 98]
QROWS = [3200, 3200, 3072, 3072]   # rows per core per quarter
QROW0 = [0, 3200, 6400, 9472]
QTBASE = [0, 25600, 51200, 75776]  # quarter-major table base (8*QROWS cumsum)
NPAD = 100352


def _np_fp8():
    import ml_dtypes
    return ml_dtypes.float8_e4m3


def _build_bass(K, NCH, choff, weights, biases, Wp, bp):
    """K[b]: chunks per dest block; NCH total chunks; choff[b] first chunk col."""
    from concourse import bass, bacc, mybir
    import concourse.tile as tile

    nc = bacc.Bacc(num_devices=NCORES, num_swdge_queues=4)
    bf = mybir.dt.bfloat16
    f32 = mybir.dt.float32
    f8 = mybir.dt.float8e4

    x_in = nc.declare_dram_parameter("x_in", [SLICE, F], f8, isOutput=False)
    midx_in = nc.declare_dram_parameter("midx", [P, NCH], mybir.dt.int32, isOutput=False)
    mdlc_in = nc.declare_dram_parameter("mdlc", [P, NCH], f32, isOutput=False)
    disq_in = nc.declare_dram_parameter("disq", [P, NBLK], f32, isOutput=False)
    sidx_in = nc.declare_dram_parameter("sidx", [P, NBLK], mybir.dt.int32, isOutput=False)
    y_out = nc.declare_dram_parameter("y_out", [P, 256], f32, isOutput=True)

    # staging + quarter-major tables
    xq = [nc.dram_tensor(f"xq{q}", [QROWS[q], F], f8) for q in range(4)]
    h_q = [[nc.dram_tensor(f"hq{i}_{q}", [QROWS[q], F], f8) for q in range(4)]
           for i in range(3)]
    vt = [nc.dram_tensor(f"vt{i}", [NPAD, F], f8, addr_space="Shared")
          for i in range(4)]

    iota_np = np.broadcast_to(np.arange(P, dtype=np.float32), (P, P)).copy()
    import ml_dtypes
    iota_d = nc.inline_tensor(iota_np.astype(ml_dtypes.bfloat16), name="iota_c")
    ident_d = nc.inline_tensor(np.eye(P, dtype=np.float32).astype(_np_fp8()), name="ident_c")
    W_d = [nc.inline_tensor(np.ascontiguousarray(w.astype(ml_dtypes.bfloat16)), name=f"W{i}")
           for i, w in enumerate(weights)]
    B_d = [nc.inline_tensor(np.broadcast_to(b.astype(np.float32), (P, F)).copy(), name=f"B{i}")
           for i, b in enumerate(biases)]
    Wp_d = [nc.inline_tensor(np.ascontiguousarray(Wp[i * F:(i + 1) * F, :].astype(ml_dtypes.bfloat16)), name=f"Wp{i}")
            for i in range(3)]
    bpd = float(bp[0] - bp[1])
    bpd_d = nc.inline_tensor(np.full((P, 1), bpd, np.float32), name="bpd_c")

    AF = mybir.ActivationFunctionType
    ALU = mybir.AluOpType
    rg = [list(range(NCORES))]

    with tile.TileContext(nc) as tc:
        with (
            tc.tile_pool(name="const", bufs=1) as cpool,
            tc.tile_pool(name="msg", bufs=8) as mpool,
            tc.tile_pool(name="work", bufs=4) as pool,
            tc.tile_pool(name="epi", bufs=1) as epool,
            tc.tile_pool(name="psum", bufs=2, space="PSUM") as psum,
            tc.tile_pool(name="psum2", bufs=2, space="PSUM") as psum2,
        ):
            # prologue: stage x quarters -> round-0 table
            for q in range(4):
                nc.sync.dma_start(out=xq[q][:, :], in_=x_in[QROW0[q]:QROW0[q] + QROWS[q], :])
                nc.gpsimd.collective_compute(
                    "AllGather", ALU.bypass, replica_groups=rg,
                    ins=[xq[q][:].opt()],
                    outs=[vt[0][QTBASE[q]:QTBASE[q] + 8 * QROWS[q], :].opt()],
                )

            iota_t = cpool.tile([P, P], bf)
            nc.sync.dma_start(out=iota_t[:], in_=iota_d[:, :])
            ident_t = cpool.tile([P, P], f8)
            nc.sync.dma_start(out=ident_t[:], in_=ident_d[:, :])
            bpd_t = cpool.tile([P, 1], f32)
            nc.sync.dma_start(out=bpd_t[:], in_=bpd_d[:, :])
            W_t, B_t, Wp_t = [], [], []
            for i in range(3):
                wt = cpool.tile([P, F], bf, tag=f"w{i}")
                nc.sync.dma_start(out=wt[:], in_=W_d[i][:, :])
                W_t.append(wt)
                bt = cpool.tile([P, F], f32, tag=f"b{i}")
                nc.sync.dma_start(out=bt[:], in_=B_d[i][:, :])
                B_t.append(bt)
                wpt = cpool.tile([P, 2], bf, tag=f"wp{i}")
                nc.sync.dma_start(out=wpt[:], in_=Wp_d[i][:, :])
                Wp_t.append(wpt)

            midx_t = cpool.tile([P, NCH], mybir.dt.int32)
            nc.sync.dma_start(out=midx_t[:], in_=midx_in[:, :])
            mdlc_t = cpool.tile([P, NCH], f32)
            nc.sync.dma_start(out=mdlc_t[:], in_=mdlc_in[:, :])
            disq_t = cpool.tile([P, NBLK], f32)
            nc.sync.dma_start(out=disq_t[:], in_=disq_in[:, :])
            sidx_t = cpool.tile([P, NBLK], mybir.dt.int32)
            nc.sync.dma_start(out=sidx_t[:], in_=sidx_in[:, :])

            yA = cpool.tile([P, 256], f32)
            nc.vector.memset(yA[:], 0.0)
            hsl = cpool.tile([P, SLICE], f8)

            for i in range(4):
                for b in range(NBLK):
                    gt = psum.tile([P, P], f32, tag="gt", space="PSUM")
                    kb = K[b]
                    for k in range(kb):
                        col = choff[b] + k
                        msg = mpool.tile([P, F], f8, tag="msg")
                        gins = nc.gpsimd.indirect_dma_start(
                            out=msg[:], out_offset=None,
                            in_=vt[i][:],
                            in_offset=bass.IndirectOffsetOnAxis(
                                ap=midx_t[:, col:col + 1], axis=0),
                        )
                        qn = col % 4
                        gins.ins.queue = f"qPoolDynamic{qn or ''}"
                        S = pool.tile([P, P], f8, tag="S")
                        nc.vector.tensor_scalar(
                            out=S[:], in0=iota_t[:],
                            scalar1=mdlc_t[:, col:col + 1], op0=ALU.is_equal,
                            scalar2=1.0, op1=ALU.mult,
                        )
                        nc.tensor.matmul(out=gt[:], lhsT=msg[:], rhs=S[:],
                                         start=(k == 0), stop=False)
                    # self-loop: gt += hsl_b^T (table holds dis*h; self norm =
                    # dis_d * (dis*h)_d handled by the outer dis_d scale)
                    if i > 0:
                        nc.tensor.matmul(out=gt[:], lhsT=hsl[:, b * P:(b + 1) * P],
                                         rhs=ident_t[:], start=False, stop=True)
                    else:
                        xb = mpool.tile([P, F], f8, tag="msg")
                        gins = nc.gpsimd.indirect_dma_start(
                            out=xb[:], out_offset=None, in_=vt[0][:],
                            in_offset=bass.IndirectOffsetOnAxis(
                                ap=sidx_t[:, b:b + 1], axis=0),
                        )
                        gins.ins.queue = f"qPoolDynamic{(b % 4) or ''}"
                        nc.tensor.matmul(out=gt[:], lhsT=xb[:], rhs=ident_t[:],
                                         start=False, stop=True)
                    gts = pool.tile([P, P], bf, tag="gts")
                    nc.scalar.copy(out=gts[:], in_=gt[:])
                    if i < 3:
                        hp = psum2.tile([P, P], f32, tag="hp", space="PSUM")
                        nc.tensor.matmul(out=hp[:], lhsT=gts[:], rhs=W_t[i][:],
                                         start=True, stop=True)
                        hb = hsl[:, b * P:(b + 1) * P]
                        tmp = pool.tile([P, P], f32, tag="tmp")
                        nc.vector.tensor_scalar(out=tmp[:], in0=hp[:],
                                                scalar1=disq_t[:, b:b + 1], op0=ALU.mult,
                                                scalar2=0.0, op1=ALU.add)
                        nc.vector.tensor_tensor(out=tmp[:], in0=tmp[:], in1=B_t[i][:], op=ALU.add)
                        nc.vector.tensor_scalar(out=hb, in0=tmp[:],
                                                scalar1=0.0, op0=ALU.max,
                                                scalar2=disq_t[:, b:b + 1], op1=ALU.mult)
                    if i >= 1:
                        ypT = psum2.tile([P, 2], f32, tag="ypT", space="PSUM")
                        nc.tensor.matmul(out=ypT[:], lhsT=gts[:], rhs=Wp_t[i - 1][:],
                                         start=True, stop=True)
                        yps = pool.tile([P, 2], f32, tag="yps")
                        nc.vector.tensor_scalar(out=yps[:], in0=ypT[:],
                                                scalar1=disq_t[:, b:b + 1], op0=ALU.mult,
                                                scalar2=0.0, op1=ALU.add)
                        nc.vector.tensor_tensor(out=yA[:, b:b + 1], in0=yA[:, b:b + 1],
                                                in1=yps[:, 0:1], op=ALU.add)
                        nc.vector.tensor_tensor(out=yA[:, 128 + b:129 + b], in0=yA[:, 128 + b:129 + b],
                                                in1=yps[:, 1:2], op=ALU.add)
                    if i < 3:
                        for q in range(4):
                            if b == QB0[q + 1] - 1:
                                nb_q = QROWS[q] // P
                                c0 = QB0[q] * P
                                nc.sync.dma_start(
                                    out=h_q[i][q][:, :].rearrange("(b d) o -> d b o", d=P),
                                    in_=hsl[:, c0:c0 + nb_q * P].rearrange("d (b o) -> d b o", o=P))
                                nc.gpsimd.collective_compute(
                                    "AllGather", ALU.bypass, replica_groups=rg,
                                    ins=[h_q[i][q][:].opt()],
                                    outs=[vt[i + 1][QTBASE[q]:QTBASE[q] + 8 * QROWS[q], :].opt()],
                                )

            dif = epool.tile([P, NBLK], f32, tag="dif")
            nc.vector.tensor_tensor(out=dif[:], in0=yA[:, 0:NBLK], in1=yA[:, 128:128 + NBLK],
                                    op=ALU.subtract)
            sig = epool.tile([P, NBLK], f32, tag="sig")
            nc.scalar.activation(out=sig[:], in_=dif[:], func=AF.Sigmoid, bias=bpd_t[:])
            om = epool.tile([P, NBLK], f32, tag="om")
            nc.vector.tensor_scalar(out=om[:], in0=sig[:],
                                    scalar1=-1.0, op0=ALU.mult,
                                    scalar2=1.0, op1=ALU.add)
            nc.sync.dma_start(out=y_out[:, 0:NBLK], in_=sig[:])
            nc.sync.dma_start(out=y_out[:, 128:128 + NBLK], in_=om[:])

    nc.compile()
    return nc


def _prep(x, edge_index):
    """Single dest-block-sorted edge stream (self-loops excluded; handled by
    identity matmul), int32 quarter-major table indices, dis scales."""
    fp8 = _np_fp8()
    N = x.shape[0]
    row = edge_index[0].astype(np.int64)
    col = edge_index[1].astype(np.int64)
    deg = np.bincount(col, minlength=N).astype(np.float32) + 1.0   # + self loop
    dis = np.zeros(NPAD, np.float32)
    dis[:N] = 1.0 / np.sqrt(deg)

    # exclude accidental self-edges in the random edge list? reference keeps
    # them as normal edges; only the explicit added self-loop is special.
    qi = np.repeat(np.arange(4), QBLK)
    QR = np.array(QROWS); Q0 = np.array(QROW0); QT = np.array(QTBASE)

    c_d = col // SLICE
    s_d = col % SLICE
    bl = s_d // P
    dlc = (s_d % P).astype(np.float32)
    c_s = row // SLICE
    s_s = row % SLICE
    qs = qi[s_s // P]
    loc = (QT[qs] + c_s * QR[qs] + s_s - Q0[qs]).astype(np.int32)

    key = c_d * NBLK + bl
    order = np.argsort(key, kind='stable')
    key_s = key[order]; loc_s = loc[order]; dlc_s = dlc[order]
    nrm_s = (dis[row] * dis[col])[order]

    ngrp = NCORES * NBLK
    cnt = np.bincount(key_s, minlength=ngrp).reshape(NCORES, NBLK)
    K = np.ceil(cnt.max(axis=0) / P).astype(np.int64)      # unified per block
    gstart = np.zeros(ngrp + 1, np.int64)
    np.cumsum(cnt.reshape(-1), out=gstart[1:])
    rank = np.arange(key_s.shape[0], dtype=np.int64) - gstart[key_s]

    choff = np.zeros(NBLK, np.int64)
    choff[1:] = np.cumsum(K[:-1])
    NCH = int(K.sum())
    L = NCH * P

    # per-edge norm must be folded into S? S is 0/1 now; fold dis into table
    # and dest scale. messages need norm = dis_s*dis_d; table holds dis_s*h;
    # outer dis_d applied post-agg => per-edge S weight is exactly 1. OK.
    pos = choff[bl[order]] * P + rank

    metas = []
    for c in range(NCORES):
        msk = (key_s // NBLK) == c
        idxs = np.tile(np.arange(P, dtype=np.int32), L // P)  # pad: rows 0..127
        dl = np.zeros(L, np.float32)
        p2 = pos[msk]
        idxs[p2] = loc_s[msk]
        dl[p2] = dlc_s[msk]
        # pad slots: dlc 0 but idx harmless; S column 0 only gets real pad
        # contributions? pad dlc=0 -> S[p,0]=1 would ADD pad rows to dest 0!!
        # fix: pad dlc = -1 -> never equals iota (0..127) -> S row all zero.
        pads = np.ones(L, bool)
        pads[p2] = False
        dl[pads] = -1.0
        midx = idxs.reshape(NCH, P).T.copy()               # [128, NCH]
        mdlc = dl.reshape(NCH, P).T.copy()
        disq = dis[c * SLICE + np.arange(SLICE)].reshape(NBLK, P).T.copy()  # [128, NBLK]
        s_all = np.arange(SLICE, dtype=np.int64)
        qsb = qi[s_all // P]
        sidx = (QT[qsb] + c * QR[qsb] + s_all - Q0[qsb]).astype(np.int32)
        sidx = sidx.reshape(NBLK, P).T.copy()              # [128, NBLK]
        metas.append((midx.astype(np.int32), mdlc.astype(np.float32),
                      np.ascontiguousarray(disq), np.ascontiguousarray(sidx)))

    x_pad = np.zeros((NCORES, SLICE, F), np.float32)
    xr = x.astype(np.float32) * dis[:N, None]
    for c in range(NCORES):
        lo = c * SLICE
        hi = min((c + 1) * SLICE, N)
        x_pad[c, :hi - lo] = xr[lo:hi]
    x_f8 = x_pad.astype(fp8)

    return metas, x_f8, K, NCH, choff


LAST_RESULTS = None
LAST_NC = None
LAST_IN_MAPS = None


def kernel(x, edge_index, W0, b0, W1, b1, W2, b2, Wp, bp):
    global LAST_RESULTS, LAST_NC, LAST_IN_MAPS
    import os
    from concourse.bass_utils import run_bass_kernel_spmd

    x = np.asarray(x, dtype=np.float32)
    edge_index = np.asarray(edge_index)
    N = x.shape[0]

    metas, x_f8, K, NCH, choff = _prep(x, edge_index)

    nc = _build_bass(
        K, NCH, choff,
        [np.asarray(W0), np.asarray(W1), np.asarray(W2)],
        [np.asarray(b0), np.asarray(b1), np.asarray(b2)],
        np.asarray(Wp), np.asarray(bp),
    )

    in_maps = []
    for c in range(NCORES):
        midx, mdlc, disq, sidx = metas[c]
        in_maps.append({"x_in": x_f8[c], "midx": midx, "mdlc": mdlc,
                        "disq": disq, "sidx": sidx})

    trace = bool(os.environ.get("KERNEL_TRACE"))
    res = run_bass_kernel_spmd(nc, in_maps, list(range(NCORES)), trace=trace)
    LAST_RESULTS = res
    LAST_NC = nc
    LAST_IN_MAPS = in_maps

    out = np.zeros((NCORES * SLICE, 2), np.float32)
    for c in range(NCORES):
        yo = res.results[c]["y_out"]
        out[c * SLICE:(c + 1) * SLICE, 0] = yo[:, 0:NBLK].T.reshape(SLICE)
        out[c * SLICE:(c + 1) * SLICE, 1] = yo[:, 128:128 + NBLK].T.reshape(SLICE)
    return out[:N]
